# revision 20
# baseline (speedup 1.0000x reference)
"""Bidirectional-ALiBi bias kernel for Trainium2 (Bass/Tile), 8-core SPMD.

Computes out[h, i, j] = |j - i| * m where m = alpha[h] on the first
row/column, gamma[h] above the diagonal, beta[h] below it, and 0 on the
(non-edge) diagonal.  Output [16, 2048, 2048] f32, sharded 2 heads/core.

Strategy ("zero-copy window DMA"): every interior row i is a shifted
window of the per-head profile V(k) = gamma*max(k,0) + beta*max(-k,0),
k = j - i.  Instead of assembling each 128-row block in a private SBUF
tile (v1: 2-3 vector/scalar copies per block gated the DMA stream until
~30us), each head keeps two profile images and blocks are DMA'd
STRAIGHT out of them with shifted column windows:

  Whi[p, cc] = V(cc - p)           cc in [0, 2048)   (k >= -127)
  Wlo[p, u]  = V(u - 1920 - p)     u  in [0, 1920)   (k < 0 tail)

Block t covers out rows [128t, 128t+128): cols [128t, 2048) stream
STRAIGHT from Whi[:, 0:2048-128t] (zero compute, 77% of all bytes);
cols [0, 128t) ship from per-block left tiles [alpha*i col-0 patch |
V-cols from Wlo] assembled in a rotating pool on the otherwise-idle
vector engine; block 0 is one patched contiguous 1-MiB tile.  head0
rides the sync HWDGE ring, head1 the scalar one.  Lessons baked in
(each measured the hard way): every bulk DMA uses all 128 partitions
(a 127-partition window DMA is not split across the 16 SDMA engines --
it lands on ONE engine at ~12 GB/s and clogs that engine's FIFO plus a
completion-semaphore lane for ~80us); left tiles are write-once/read-
once pool buffers (appending into a shared image makes every reader
DMA a WAR blocker for the next build -- Tile deps are tile-granular --
serializing build<->DMA into a 25us tail); big and small pieces are
issue-order PAIRED so the ~4 in-flight DMAs a ring gets from the 8
shared completion-semaphore lanes always hold enough bytes to cover
the drain rate (a run of small pieces dips the stream to ~200 GB/s).

Ramp fixes vs v1: the 6 coefficients (alpha, gamma, and HOST-computed
slope -beta/gamma per head) are packed into ONE [1,6] dram tensor ->
one single-descriptor DMA (v1's three 128-descriptor partition
broadcasts took ~8.8us to land), then broadcast to all 128 partitions
with a 1x128-ones PE matmul through PSUM.  First block DMA needs only
coef + one PRelu ACTIVATE over Khi: bytes start flowing ~14us in and
the 16-SDMA wall (~425-435 GB/s) carries the whole 33.5 MiB stream.
"""

import numpy as np

H = 16
S = 2048
P = 128
N_CORES = 8
H_LOC = H // N_CORES  # 2 heads per core
NT = S // P  # 16 row blocks per head
W_LO = S - P  # 1920: lo image covers k in [-1920-p, -p)

_NC = None


def _build():
    import concourse.bacc as bacc
    import concourse.mybir as mybir
    from concourse.tile import TileContext

    f32 = mybir.dt.float32
    nc = bacc.Bacc("TRN2", target_bir_lowering=False, debug=False)

    # coef = [a0, a1, g0, g1, s0, s1], s = -beta/gamma (computed on host)
    coef_d = nc.dram_tensor("coef", [1, 6], f32, kind="ExternalInput").ap()
    out_d = nc.dram_tensor("out", [H_LOC, S, S], f32, kind="ExternalOutput").ap()

    with TileContext(nc) as tc:
        with (
            tc.tile_pool(name="mem", bufs=1) as mem,
            tc.tile_pool(name="lpool", bufs=12) as lpool,
            tc.tile_pool(name="pp", bufs=1, space="PSUM") as pp,
        ):
            # --- coefficient load: one tiny contiguous DMA, first thing issued
            coefT = mem.tile([1, 6], f32, name="coefT")
            nc.sync.dma_start(out=coefT[:], in_=coef_d)

            ones = mem.tile([1, P], f32, name="ones")
            nc.vector.memset(ones[:], 1.0)

            # --- iotas (gpsimd, overlap the coef DMA + preamble)
            def iota(name, width, base, mult, pattern=None):
                Kt = mem.tile([P, width], f32, name=name)
                nc.gpsimd.iota(
                    Kt[:],
                    pattern=pattern or [[1, width]],
                    base=base,
                    channel_multiplier=mult,
                    allow_small_or_imprecise_dtypes=True,
                )
                return Kt

            # Khi[p, cc] = cc - p, split in halves: the [128,2048] iota alone
            # took 3.5us on gpsimd and sat on the first-byte critical path;
            # the left half (all the first DMAs need) is ready in ~1.8us,
            # before the coefficient DMA even lands.
            HB = S // 2  # 1024
            KhiL = iota("KhiL", HB, 0, -1)  # cc in [0, 1024)
            KhiR = iota("KhiR", HB, HB, -1)  # cc in [1024, 2048)
            IB = iota("IB", NT, 0, 1, pattern=[[P, NT]])  # IB[p, t] = 128t + p
            Klo = iota("Klo", W_LO, -W_LO, -1)  # Klo[p, u] = u - 1920 - p

            # --- broadcast coef to all partitions via 1x128-ones matmul
            Cp = pp.tile([P, 6], f32, name="Cp")
            nc.tensor.matmul(Cp[:], ones[:], coefT[:])
            C = mem.tile([P, 6], f32, name="C")
            nc.vector.tensor_copy(out=C[:], in_=Cp[:])

            def A(h):  # alpha[h], per-partition
                return C[:, h : h + 1]

            def G(h):  # gamma[h]
                return C[:, 2 + h : 3 + h]

            def SL(h):  # -beta[h]/gamma[h]
                return C[:, 4 + h : 5 + h]

            # --- profile images: V(k) = PRelu(gamma*k) with slope -beta/gamma,
            # Whi split in halves so the first (half-width, ~1.05us) ACTIVATE
            # unblocks the stream ~1.5us earlier than a full-width one would
            def prelu(out, in_, h):
                nc.scalar.activation(
                    out=out,
                    in_=in_,
                    func=mybir.ActivationFunctionType.Prelu,
                    scale=G(h),
                    alpha=SL(h),
                )

            WhiL = [mem.tile([P, HB], f32, name=f"WhiL{h}") for h in range(H_LOC)]
            WhiR = [mem.tile([P, HB], f32, name=f"WhiR{h}") for h in range(H_LOC)]
            Wlo = [mem.tile([P, W_LO], f32, name=f"Wlo{h}") for h in range(H_LOC)]
            for h in range(H_LOC):
                prelu(WhiL[h][:], KhiL[:], h)
            for h in range(H_LOC):
                prelu(WhiR[h][:], KhiR[:], h)
            for h in range(H_LOC):
                prelu(Wlo[h][:], Klo[:], h)

            # --- block 0 as one fully-contiguous 1-MiB tile: all 2048 cols
            # copied from Whi, then col 0 <- alpha*p and row 0 <- alpha*j
            # (program order makes the patches win).  Full 128 partitions:
            # compute ops must start at partition 0, and a 127-partition DMA
            # is not split across the 16 SDMA engines (it lands on ONE
            # engine at ~12 GB/s and clogs that engine's FIFO + a semaphore
            # lane for ~80us -- the v2 failure mode).
            T0 = [mem.tile([P, S], f32, name=f"T0_{h}") for h in range(H_LOC)]
            for h in range(H_LOC):
                nc.vector.tensor_copy(out=T0[h][:, 0:HB], in_=WhiL[h][:])
                nc.vector.tensor_copy(out=T0[h][:, HB:S], in_=WhiR[h][:])
                nc.vector.tensor_scalar_mul(T0[h][:, 0:1], IB[:, 0:1], A(h))
                nc.vector.tensor_scalar_mul(
                    T0[h][0:1, 0:HB], KhiL[0:1, :], C[0:1, h : h + 1]
                )
                nc.vector.tensor_scalar_mul(
                    T0[h][0:1, HB:S], KhiR[0:1, :], C[0:1, h : h + 1]
                )

            # --- left pieces, cols [0, 128t) of block t: col 0 = alpha*i
            # patch + V-columns from Wlo, assembled in a rotating pool
            # (write-once/read-once tiles; a shared per-head image would make
            # every strip DMA a WAR blocker for the next build -- Tile
            # dependencies are tile-granular -- serializing build<->DMA at
            # ~4us each, measured as a 25us tail).  Emission (= issue) order
            # pairs big and small pieces so the ~4 in-flight DMAs a ring
            # gets from the 8 shared completion-semaphore lanes always hold
            # >~2 MiB: a run of small pieces drops in-flight bytes below the
            # drain rate and the stream dips (measured 196-302 GB/s).
            def pair_order(ts):
                # [biggest, smallest, 2nd-biggest, 2nd-smallest, ...]
                out, lo_i, hi_i = [], 0, len(ts) - 1
                while lo_i <= hi_i:
                    out.append(ts[hi_i])
                    if lo_i != hi_i:
                        out.append(ts[lo_i])
                    hi_i -= 1
                    lo_i += 1
                return out

            left_order = pair_order(list(range(1, NT)))  # [15,1,14,2,...]
            Ls = {}
            for t in left_order:
                for h in range(H_LOC):
                    L = lpool.tile([P, W_LO], f32, tag="L", name=f"L{h}_{t}")
                    Ls[(h, t)] = L
                    w = P * t
                    nc.vector.tensor_scalar_mul(L[:, 0:1], IB[:, t : t + 1], A(h))
                    nc.vector.tensor_copy(
                        out=L[:, 1:w], in_=Wlo[h][:, W_LO + 1 - w : W_LO]
                    )

            # --- block streams: head0 on sync ring, head1 on scalar ring
            rings = [nc.sync, nc.scalar]
            for h in range(H_LOC):
                r = rings[h]
                # A-halves of t=1..7 first: full-WhiL windows (0.5 MiB each),
                # issuable right after the half-width PRelu
                for t in range(1, 8):
                    r.dma_start(
                        out=out_d[h, P * t : P * t + P, P * t : P * t + HB],
                        in_=WhiL[h][:],
                    )

                def b_piece(t):  # cols [128t+1024, 2048) from WhiR
                    r.dma_start(
                        out=out_d[h, P * t : P * t + P, P * t + HB : S],
                        in_=WhiR[h][:, 0 : HB - P * t],
                    )

                def far_piece(t):  # t >= 8: whole window inside WhiL
                    r.dma_start(
                        out=out_d[h, P * t : P * t + P, P * t : S],
                        in_=WhiL[h][:, 0 : S - P * t],
                    )

                # B-halves + far windows descending; 1-MiB block 0 mid-phase
                for bt, ft in ((1, 8), (2, 9), (3, 10)):
                    b_piece(bt)
                    far_piece(ft)
                r.dma_start(out=out_d[h, 0:P, :], in_=T0[h][:])
                for bt, ft in ((4, 11), (5, 12), (6, 13)):
                    b_piece(bt)
                    far_piece(ft)
                b_piece(7)
                far_piece(14)
                far_piece(15)
                for t in left_order:
                    r.dma_start(
                        out=out_d[h, P * t : P * t + P, 0 : P * t],
                        in_=Ls[(h, t)][:, 0 : P * t],
                    )

    nc.compile()
    return nc


def _run(alpha, beta, gamma, **spmd_kwargs):
    """Compile (cached) and run on the 8 NeuronCores; returns BassKernelResults."""
    global _NC
    if _NC is None:
        _NC = _build()
    from concourse import bass_utils

    alpha = np.ascontiguousarray(alpha, dtype=np.float32)
    beta = np.ascontiguousarray(beta, dtype=np.float32)
    gamma = np.ascontiguousarray(gamma, dtype=np.float32)
    in_maps = []
    for c in range(N_CORES):
        sl = slice(c * H_LOC, (c + 1) * H_LOC)
        coef = np.concatenate(
            [alpha[sl], gamma[sl], -beta[sl] / gamma[sl]]
        ).astype(np.float32).reshape(1, 6)
        in_maps.append({"coef": coef})
    return bass_utils.run_bass_kernel_spmd(
        _NC, in_maps, core_ids=list(range(N_CORES)), **spmd_kwargs
    )


def kernel(alpha, beta, gamma, seq_len):
    assert int(seq_len) == S, f"kernel hardcodes seq_len={S}, got {seq_len}"
    res = _run(alpha, beta, gamma)
    return np.concatenate([r["out"] for r in res.results], axis=0)


# revision 21
# speedup vs baseline: 1.0595x; 1.0595x over previous
"""Bidirectional-ALiBi bias kernel for Trainium2 (Bass/Tile), 8-core SPMD.

Computes out[h, i, j] = |j - i| * m where m = alpha[h] on the first
row/column, gamma[h] above the diagonal, beta[h] below it, and 0 on the
(non-edge) diagonal.  Output [16, 2048, 2048] f32, sharded 2 heads/core.

Strategy ("zero-copy window DMA"): every interior row i is a shifted
window of the per-head profile V(k) = gamma*max(k,0) + beta*max(-k,0),
k = j - i.  Instead of assembling each 128-row block in a private SBUF
tile (v1: 2-3 vector/scalar copies per block gated the DMA stream until
~30us), each head keeps two profile images and blocks are DMA'd
STRAIGHT out of them with shifted column windows:

  Whi[p, cc] = V(cc - p)           cc in [0, 2048)   (k >= -127)
  Wlo[p, u]  = V(u - 1920 - p)     u  in [0, 1920)   (k < 0 tail)

Block t covers out rows [128t, 128t+128): cols [128t, 2048) stream
STRAIGHT from Whi[:, 0:2048-128t] (zero compute, 77% of all bytes);
cols [0, 128t) ship from per-block left tiles [alpha*i col-0 patch |
V-cols from Wlo] assembled in a rotating pool on the otherwise-idle
vector engine; block 0 is one patched contiguous 1-MiB tile.  head0
rides the sync HWDGE ring, head1 the scalar one.  Lessons baked in
(each measured the hard way): every bulk DMA uses all 128 partitions
(a 127-partition window DMA is not split across the 16 SDMA engines --
it lands on ONE engine at ~12 GB/s and clogs that engine's FIFO plus a
completion-semaphore lane for ~80us); left tiles are write-once/read-
once pool buffers (appending into a shared image makes every reader
DMA a WAR blocker for the next build -- Tile deps are tile-granular --
serializing build<->DMA into a 25us tail); big and small pieces are
issue-order PAIRED so the ~4 in-flight DMAs a ring gets from the 8
shared completion-semaphore lanes always hold enough bytes to cover
the drain rate (a run of small pieces dips the stream to ~200 GB/s).

Ramp fixes vs v1: the 6 coefficients (alpha, gamma, and HOST-computed
slope -beta/gamma per head) are packed into ONE [1,6] dram tensor ->
one single-descriptor DMA (v1's three 128-descriptor partition
broadcasts took ~8.8us to land), then broadcast to all 128 partitions
with a 1x128-ones PE matmul through PSUM.  First block DMA needs only
coef + one PRelu ACTIVATE over Khi: bytes start flowing ~14us in and
the 16-SDMA wall (~425-435 GB/s) carries the whole 33.5 MiB stream.
"""

import numpy as np

H = 16
S = 2048
P = 128
N_CORES = 8
H_LOC = H // N_CORES  # 2 heads per core
NT = S // P  # 16 row blocks per head
W_LO = S - P  # 1920: lo image covers k in [-1920-p, -p)

_NC = None


def _build():
    import concourse.bacc as bacc
    import concourse.mybir as mybir
    from concourse.tile import TileContext

    f32 = mybir.dt.float32
    nc = bacc.Bacc("TRN2", target_bir_lowering=False, debug=False)

    # coef = [a0, a1, g0, g1, s0, s1], s = -beta/gamma (computed on host)
    coef_d = nc.dram_tensor("coef", [1, 6], f32, kind="ExternalInput").ap()
    out_d = nc.dram_tensor("out", [H_LOC, S, S], f32, kind="ExternalOutput").ap()

    with TileContext(nc) as tc:
        with (
            tc.tile_pool(name="mem", bufs=1) as mem,
            tc.tile_pool(name="lpool", bufs=12) as lpool,
            tc.tile_pool(name="pp", bufs=1, space="PSUM") as pp,
        ):
            # --- coefficient load: one tiny contiguous DMA, first thing issued
            coefT = mem.tile([1, 6], f32, name="coefT")
            nc.sync.dma_start(out=coefT[:], in_=coef_d)

            ones = mem.tile([1, P], f32, name="ones")
            nc.vector.memset(ones[:], 1.0)

            # --- iotas (gpsimd, overlap the coef DMA + preamble)
            def iota(name, width, base, mult, pattern=None):
                Kt = mem.tile([P, width], f32, name=name)
                nc.gpsimd.iota(
                    Kt[:],
                    pattern=pattern or [[1, width]],
                    base=base,
                    channel_multiplier=mult,
                    allow_small_or_imprecise_dtypes=True,
                )
                return Kt

            Khi = iota("Khi", S, 0, -1)  # Khi[p, cc] = cc - p
            IB = iota("IB", NT, 0, 1, pattern=[[P, NT]])  # IB[p, t] = 128t + p
            Klo = iota("Klo", W_LO, -W_LO, -1)  # Klo[p, u] = u - 1920 - p

            # --- broadcast coef to all partitions via 1x128-ones matmul
            Cp = pp.tile([P, 6], f32, name="Cp")
            nc.tensor.matmul(Cp[:], ones[:], coefT[:])
            C = mem.tile([P, 6], f32, name="C")
            nc.vector.tensor_copy(out=C[:], in_=Cp[:])

            def A(h):  # alpha[h], per-partition
                return C[:, h : h + 1]

            def G(h):  # gamma[h]
                return C[:, 2 + h : 3 + h]

            def SL(h):  # -beta[h]/gamma[h]
                return C[:, 4 + h : 5 + h]

            # --- profile images: V(k) = PRelu(gamma*k) with slope -beta/gamma
            Whi = [mem.tile([P, S], f32, name=f"Whi{h}") for h in range(H_LOC)]
            Wlo = [mem.tile([P, W_LO], f32, name=f"Wlo{h}") for h in range(H_LOC)]
            for h in range(H_LOC):
                nc.scalar.activation(
                    out=Whi[h][:],
                    in_=Khi[:],
                    func=mybir.ActivationFunctionType.Prelu,
                    scale=G(h),
                    alpha=SL(h),
                )
            for h in range(H_LOC):
                nc.scalar.activation(
                    out=Wlo[h][:],
                    in_=Klo[:],
                    func=mybir.ActivationFunctionType.Prelu,
                    scale=G(h),
                    alpha=SL(h),
                )

            # --- block 0 as one fully-contiguous 1-MiB tile: all 2048 cols
            # copied from Whi, then col 0 <- alpha*p and row 0 <- alpha*j
            # (program order makes the patches win).  Full 128 partitions:
            # compute ops must start at partition 0, and a 127-partition DMA
            # is not split across the 16 SDMA engines (it lands on ONE
            # engine at ~12 GB/s and clogs that engine's FIFO + a semaphore
            # lane for ~80us -- the v2 failure mode).
            T0 = [mem.tile([P, S], f32, name=f"T0_{h}") for h in range(H_LOC)]
            for h in range(H_LOC):
                nc.vector.tensor_copy(out=T0[h][:], in_=Whi[h][:])
                nc.vector.tensor_scalar_mul(T0[h][:, 0:1], IB[:, 0:1], A(h))
                nc.vector.tensor_scalar_mul(
                    T0[h][0:1, :], Khi[0:1, :], C[0:1, h : h + 1]
                )

            # --- left pieces, cols [0, 128t) of block t: col 0 = alpha*i
            # patch + V-columns from Wlo, assembled in a rotating pool
            # (write-once/read-once tiles; a shared per-head image would make
            # every strip DMA a WAR blocker for the next build -- Tile
            # dependencies are tile-granular -- serializing build<->DMA at
            # ~4us each, measured as a 25us tail).  Emission (= issue) order
            # pairs big and small pieces so the ~4 in-flight DMAs a ring
            # gets from the 8 shared completion-semaphore lanes always hold
            # >~2 MiB: a run of small pieces drops in-flight bytes below the
            # drain rate and the stream dips (measured 196-302 GB/s).
            def pair_order(ts):
                # [biggest, smallest, 2nd-biggest, 2nd-smallest, ...]
                out, lo_i, hi_i = [], 0, len(ts) - 1
                while lo_i <= hi_i:
                    out.append(ts[hi_i])
                    if lo_i != hi_i:
                        out.append(ts[lo_i])
                    hi_i -= 1
                    lo_i += 1
                return out

            left_order = pair_order(list(range(1, NT)))  # [15,1,14,2,...]
            hi_order = [NT - t for t in left_order]  # [1,15,2,14,...]
            Ls = {}
            for t in left_order:
                for h in range(H_LOC):
                    L = lpool.tile([P, W_LO], f32, tag="L", name=f"L{h}_{t}")
                    Ls[(h, t)] = L
                    w = P * t
                    nc.vector.tensor_scalar_mul(L[:, 0:1], IB[:, t : t + 1], A(h))
                    nc.vector.tensor_copy(
                        out=L[:, 1:w], in_=Wlo[h][:, W_LO + 1 - w : W_LO]
                    )

            # --- block streams: head0 on sync ring, head1 on scalar ring
            rings = [nc.sync, nc.scalar]
            for h in range(H_LOC):
                r = rings[h]
                # right windows, big/small paired (t=1 is 0.94 MiB, t=15 64 KiB);
                # the 1-MiB block-0 tile rides mid-phase where the paired
                # windows run small, keeping in-flight bytes up
                for t in hi_order[:8]:
                    r.dma_start(
                        out=out_d[h, P * t : P * t + P, P * t : S],
                        in_=Whi[h][:, 0 : S - P * t],
                    )
                r.dma_start(out=out_d[h, 0:P, :], in_=T0[h][:])
                for t in hi_order[8:]:
                    r.dma_start(
                        out=out_d[h, P * t : P * t + P, P * t : S],
                        in_=Whi[h][:, 0 : S - P * t],
                    )
                for t in left_order:
                    r.dma_start(
                        out=out_d[h, P * t : P * t + P, 0 : P * t],
                        in_=Ls[(h, t)][:, 0 : P * t],
                    )

    nc.compile()
    return nc


def _run(alpha, beta, gamma, **spmd_kwargs):
    """Compile (cached) and run on the 8 NeuronCores; returns BassKernelResults."""
    global _NC
    if _NC is None:
        _NC = _build()
    from concourse import bass_utils

    alpha = np.ascontiguousarray(alpha, dtype=np.float32)
    beta = np.ascontiguousarray(beta, dtype=np.float32)
    gamma = np.ascontiguousarray(gamma, dtype=np.float32)
    in_maps = []
    for c in range(N_CORES):
        sl = slice(c * H_LOC, (c + 1) * H_LOC)
        coef = np.concatenate(
            [alpha[sl], gamma[sl], -beta[sl] / gamma[sl]]
        ).astype(np.float32).reshape(1, 6)
        in_maps.append({"coef": coef})
    return bass_utils.run_bass_kernel_spmd(
        _NC, in_maps, core_ids=list(range(N_CORES)), **spmd_kwargs
    )


def kernel(alpha, beta, gamma, seq_len):
    assert int(seq_len) == S, f"kernel hardcodes seq_len={S}, got {seq_len}"
    res = _run(alpha, beta, gamma)
    return np.concatenate([r["out"] for r in res.results], axis=0)
